# revision 63
# baseline (speedup 1.0000x reference)
"""GAT BasicAttentionBlock kernel for 8x Trainium2 NeuronCores.

Strategy (output-shard): each core owns 1250 of the 10000 selected output
rows (index0).  Only nodes reachable from those rows matter: ~1.2k unique
target nodes and ~16k unique source nodes per core (~5.5x less edge work
than the full graph).  Per core:

  phase A  gather x rows of needed nodes (host "halo"), compute
           h = relu(x@w1.T + b) feature-major on PE, then per 128-node
           subtile proj = h@w_proj.T and s_src = h@B_src node-major into a
           512-byte/row HBM table [proj bf16 128 (d-major) | s_src bf16 8 |
           garbage] (gathers below 512B/row are charged 2x, so the row pad
           is free).
           Interleaved with phase A (to fill DVE/PE/Pool gaps under the
           DMA-bound stream): the per-window reduction masks Mw, the
           expansion masks Mtw (from one big partition_broadcast), the
           s_trg edge-slot expansion (one-hot matmuls), and s_trg/skip
           for the window targets.
  loop 2   per window: dma_gather the table rows of each edge's source
           (512B granules), scores = leakyrelu(s_src+s_trg) via
           exp(lrelu(s)) = max(exp(s), exp(0.2s)), weighted = exp * proj,
           segment-sum via one-hot matmuls accumulated in PSUM:
           out[t] = [sum exp*proj | sum exp], out = att/den + skip, ELU.
  host     the final index_select (out rows -> index0 order) is a pure
           permutation of the returned per-target table; applied on host.

No collectives: cores are fully independent.  The softmax max-subtraction
in the reference cancels in the att = exp/sum(exp) ratio and is dropped
(scores are O(1) here, exp cannot overflow).
"""

import os
import sys

for _p in ("/opt/trn_rl_repo",):
    if os.path.isdir(_p) and _p not in sys.path:
        sys.path.insert(0, _p)

import numpy as np
import ml_dtypes

# problem constants (hardcoded per contract)
N = 50000
E = 800000
K = 10000
IN = 256
H = 128
NH = 8
HD = 16
OC = NH * HD  # 128
CORES = 8
KC = K // CORES          # 1250 output rows per core
P = 128
W = 10                   # target windows of 128 -> 1280 target slots
TP = W * P               # padded target count per core
EPS = 1e-16

BF16 = ml_dtypes.bfloat16


# ----------------------------------------------------------------------------
# host-side sharding / planning
# ----------------------------------------------------------------------------

def _wrap16(vals, reps=8):
    """int16 index layout for dma_gather: idx i at [i%16, i//16], the 16-row
    block replicated `reps` times down the partition axis."""
    L = vals.shape[0]
    assert L % 16 == 0
    w = vals.reshape(L // 16, 16).T.astype(np.int16)
    return np.tile(w, (reps, 1))


def plan(x, adj0, index0):
    src_all = np.asarray(adj0[0], dtype=np.int64)
    trg_all = np.asarray(adj0[1], dtype=np.int64)
    idx0 = np.asarray(index0, dtype=np.int64)
    x = np.asarray(x, dtype=np.float32)

    pre = []
    npad_req = 512
    ecs_req = np.zeros(W, np.int64)
    for c in range(CORES):
        ks = idx0[c * KC:(c + 1) * KC]
        tgt_u, inv_k = np.unique(ks, return_inverse=True)
        U_t = len(tgt_u)
        assert U_t <= TP
        lut = np.full(N, -1, np.int64)
        lut[tgt_u] = np.arange(U_t)
        tloc_all = lut[trg_all]
        sel = np.nonzero(tloc_all >= 0)[0]
        e_src = src_all[sel]
        e_tloc = tloc_all[sel]

        # balance targets across the W windows so the max per-window edge
        # count (which sets the padded slot count) is near the mean:
        # greedy first-fit-decreasing by in-degree.
        deg = np.bincount(e_tloc, minlength=U_t)
        order_t = np.argsort(-deg, kind="stable")
        wload = np.zeros(W, np.int64)
        wfill = np.zeros(W, np.int64)
        wof = np.empty(U_t, np.int64)     # target -> window
        for t in order_t:
            cand = np.nonzero(wfill < P)[0]
            wsel = cand[np.argmin(wload[cand])]
            wof[t] = wsel
            wfill[wsel] += 1
            wload[wsel] += deg[t]
        # repair pass: move small targets off overloaded windows so every
        # window fits in 16 blocks (2048 edges) when possible.
        CAP_E = 16 * P
        for _ in range(64):
            hi = int(np.argmax(wload))
            if wload[hi] <= CAP_E:
                break
            lo_ws = np.nonzero(wfill < P)[0]
            lo_ws = lo_ws[lo_ws != hi]
            if len(lo_ws) == 0:
                break
            lo = lo_ws[np.argmin(wload[lo_ws])]
            cand_t = np.nonzero(wof == hi)[0]
            need = wload[hi] - CAP_E
            fit = deg[cand_t]
            pick = cand_t[np.argmin(np.abs(fit - max(need, 1)))]
            wof[pick] = lo
            wload[hi] -= deg[pick]
            wload[lo] += deg[pick]
            wfill[hi] -= 1
            wfill[lo] += 1
        wslot = np.empty(U_t, np.int64)   # target -> window-local slot id
        fill2 = np.zeros(W, np.int64)
        for t in range(U_t):
            wsel = wof[t]
            wslot[t] = wsel * P + fill2[wsel]
            fill2[wsel] += 1

        e_tloc = wslot[e_tloc]            # re-number targets to window slots
        inv_k = wslot[inv_k]
        order = np.argsort(e_tloc, kind="stable")
        e_src = e_src[order]
        e_tloc = e_tloc[order]
        e_win = e_tloc >> 7
        cnt = np.bincount(e_win, minlength=W)
        ecs_req = np.maximum(ecs_req, (cnt + P - 1) // P)

        # halo layout: first TP rows are the target SLOTS in slot order
        # (unfilled slots duplicate tgt_u[0]), then the extra source nodes.
        slot_nodes = np.full(TP, tgt_u[0], np.int64)
        slot_nodes[wslot] = tgt_u
        extra = np.setdiff1d(np.unique(e_src), tgt_u)
        nodes = np.concatenate([slot_nodes, extra])
        npad_req = max(npad_req, len(nodes))
        pre.append((tgt_u, inv_k, e_src, e_tloc, e_win, cnt, nodes))

    ECS = tuple(int(v) for v in ecs_req)    # per-window block count
    caps = [ec * P for ec in ECS]
    coff = np.concatenate([[0], np.cumsum(caps)])   # slot offsets per window
    SLOTS = int(coff[-1])
    NPAD = ((npad_req + 511) // 512) * 512

    per_core = []
    for c in range(CORES):
        tgt_u, inv_k, e_src, e_tloc, e_win, cnt, nodes = pre[c]
        U_n = len(nodes)
        nlut = np.full(N, -1, np.int64)
        nlut[nodes] = np.arange(U_n)
        e_srcloc = nlut[e_src]

        start = np.concatenate([[0], np.cumsum(cnt)[:-1]])
        within = np.arange(len(e_tloc)) - start[e_win]
        slots = coff[e_win] + within

        esrc_flat = np.zeros(SLOTS, np.int64)
        etcol_flat = np.full(SLOTS, -1.0, np.float32)
        esrc_flat[slots] = e_srcloc
        etcol_flat[slots] = (e_tloc - e_win * P).astype(np.float32)

        # etcol: per-slot target column, slot-partition-major per window
        etcol = np.concatenate(
            [etcol_flat[coff[w]:coff[w + 1]]
             .reshape(ECS[w], P).T for w in range(W)], axis=1)  # [P, sum EC]
        etrow = etcol_flat.astype(BF16).reshape(1, SLOTS)
        eidx = np.concatenate(
            [_wrap16(esrc_flat[coff[w]:coff[w + 1]]) for w in range(W)],
            axis=1)

        xT = np.zeros((IN, NPAD), BF16)
        xT[:, :U_n] = x[nodes].T

        per_core.append(dict(xT=xT, eidx=eidx, etcol=etcol, etrow=etrow,
                             inv_k=inv_k))
    return per_core, NPAD, ECS


# d-major column permutation for proj/skip/out: new col p holds old col
# COLMAP[p] = (p % NH) * HD + (p // NH).  Makes the per-head exp broadcast a
# step-1 inner AP so the DVE multiply runs at 2x without materializing the
# expansion.  Undone on host for the final output.
COLMAP = np.array([(p % NH) * HD + (p // NH) for p in range(OC)])


def make_weights(w_in, b_in, w_proj, a_src, a_trg, w_skip):
    w_in = np.asarray(w_in, np.float32)
    b_in = np.asarray(b_in, np.float32)
    w_proj = np.asarray(w_proj, np.float32)
    a_src = np.asarray(a_src, np.float32).reshape(NH, HD)
    a_trg = np.asarray(a_trg, np.float32).reshape(NH, HD)
    w_skip = np.asarray(w_skip, np.float32)

    w1T = np.ascontiguousarray(w_in.T).astype(BF16)        # [256,128]
    b1 = b_in.reshape(H, 1).astype(np.float32)
    # B_src[h, a] = sum_d w_proj[a*16+d, h] * a_src[a, d]
    wp3 = w_proj.reshape(NH, HD, H)
    B_src = np.einsum("adh,ad->ha", wp3, a_src).astype(np.float32)  # [128,8]
    B_trg = np.einsum("adh,ad->ha", wp3, a_trg).astype(np.float32)
    w2 = np.zeros((H, 256), np.float32)
    w2[:, :OC] = w_proj.T[:, COLMAP]
    w2[:, OC:OC + NH] = B_src
    w2 = w2.astype(BF16)
    # loop1 combined rhs: [B_trg | w_skip.T (d-major)]  -> [H, NH + OC]
    btw = np.concatenate([B_trg, w_skip.T[:, COLMAP]], axis=1).astype(BF16)
    iota1 = np.arange(P, dtype=BF16).reshape(1, P).repeat(P, 0)  # [P,P]
    iota_c = np.arange(P, dtype=np.float32).reshape(P, 1)
    return dict(w1T=w1T, b1=b1, w2=w2, btw=btw, iota1=iota1, iota_c=iota_c)


# ----------------------------------------------------------------------------
# bass kernel
# ----------------------------------------------------------------------------

_BUILD_CACHE = {}


def build(NPAD, ECS):
    PARTS = int(os.environ.get("KPARTS", "5"))
    key = (NPAD, ECS, PARTS)
    if key in _BUILD_CACHE:
        return _BUILD_CACHE[key]

    import concourse.bacc as bacc
    import concourse.mybir as mybir
    import concourse.tile as tile

    dt = mybir.dt
    F32 = dt.float32
    I16 = dt.int16
    BF = dt.bfloat16
    AF = mybir.ActivationFunctionType
    OP = mybir.AluOpType

    NT = NPAD // 512
    caps = [ec * P for ec in ECS]
    coff = np.concatenate([[0], np.cumsum(caps)]).astype(int)
    SLOTS = int(coff[-1])
    SEC = sum(ECS)                       # total 128-slot blocks
    boff = np.concatenate([[0], np.cumsum(ECS)]).astype(int)
    ECMAX = max(ECS)

    nc = bacc.Bacc("TRN2", target_bir_lowering=False)

    with tile.TileContext(nc) as tc:
        with tc.tile_pool(name="dram", bufs=1, space="DRAM") as dram:
            def din(name, shape, dtp):
                return dram.tile(shape, dtp, kind="ExternalInput", name=name,
                                 uniquify=False)

            xT = din("xT", [IN, NPAD], BF)
            w1T = din("w1T", [IN, H], BF)
            b1 = din("b1", [H, 1], F32)
            w2 = din("w2", [H, 256], BF)
            btw = din("btw", [H, NH + OC], BF)
            eidx = din("eidx", [P, SLOTS // 16], I16)
            etcol = din("etcol", [P, SEC], F32)
            etrow = din("etrow", [1, SLOTS], BF)
            iota1 = din("iota1", [P, P], BF)
            iota_c = din("iota_c", [P, 1], F32)

            tabA = dram.tile([NPAD, 256], BF, kind="Internal", name="tabA",
                             uniquify=False)
            out = dram.tile([TP, OC], BF, kind="ExternalOutput", name="out",
                            uniquify=False)

        with tc.tile_pool(name="pers", bufs=1) as pers:
            w1a = pers.tile([P, H], BF)
            w1b = pers.tile([P, H], BF)
            b1s = pers.tile([H, 1], F32)
            w2s = pers.tile([H, 256], BF)
            btws = pers.tile([H, NH + OC], BF)
            iotas = pers.tile([P, P], BF)
            iotac = pers.tile([P, 1], F32)
            hfmt = pers.tile([H, TP], BF)         # targets' h, feature-major
            strg = pers.tile([P, W * NH], BF)     # per-window s_trg  [t, 8]
            skips = pers.tile([P, W, OC], BF)     # per-window skip   [t, oc]
            st_sb = pers.tile([P, SEC, NH], BF)   # s_trg per edge slot
            eidxs = pers.tile([P, SLOTS // 16], I16)
            etcols = pers.tile([P, SEC], F32)
            Mw = pers.tile([P, SEC * P], BF)      # reduction masks (all win)


            CH = 4  # 512-node tiles per xT load chunk
            with tc.tile_pool(name="pbc", bufs=1) as pbc, \
                 tc.tile_pool(name="petw", bufs=2) as petw, \
                 tc.tile_pool(name="pa", bufs=4) as pa, \
                 tc.tile_pool(name="pmtw", bufs=2) as pmtw, \
                 tc.tile_pool(name="pax", bufs=4) as pax, \
                 tc.tile_pool(name="psa", bufs=3, space="PSUM") as psa, \
                 tc.tile_pool(name="psb", bufs=3, space="PSUM") as psb, \
                 tc.tile_pool(name="psc", bufs=1, space="PSUM") as psc, \
                 tc.tile_pool(name="psd", bufs=1, space="PSUM") as psd:

                # partition-broadcast of every slot's target column, done in
                # per-window pieces interleaved with the tabA writes on
                # Pool's in-order stream (Pool is otherwise idle in phase A).
                pbcw = pbc.tile([P, SLOTS], BF, tag="pbcw")
                CAPMAX = max(caps)

                def bcast_piece(w):
                    # load just this window's slice of etrow: a [1, SLOTS]
                    # tile would cost SLOTS bytes on EVERY partition.
                    etw = petw.tile([1, CAPMAX], BF, tag="etw")
                    nc.sync.dma_start(etw[0:1, 0:caps[w]],
                                      etrow[0:1, coff[w]:coff[w + 1]])
                    nc.gpsimd.partition_broadcast(
                        pbcw[:, coff[w]:coff[w + 1]], etw[0:1, 0:caps[w]])

                # deferred work generators, interleaved under phase A below:
                # reduction masks Mw (DVE 4x) ...
                def gen_masks():
                    for k in range(SEC):
                        yield lambda k=k: nc.vector.tensor_scalar(
                            Mw[:, k * P:(k + 1) * P], iotas[:],
                            etcols[:, k:k + 1], None, OP.is_equal)
                # ... expansion masks + s_trg one-hot expansion (needs strg)
                def gen_expand():
                    for w in range(W):
                        def mk(w=w):
                            Mtw = pmtw.tile([P, ECMAX * P], BF, tag="Mtw")
                            cap = caps[w]
                            nc.vector.tensor_scalar(
                                Mtw[:, 0:cap], pbcw[:, coff[w]:coff[w + 1]],
                                iotac[:], None, OP.is_equal)
                            stp = psc.tile([P, ECMAX, NH], F32,
                                           tag="stps")
                            for j in range(ECS[w]):
                                nc.tensor.matmul(
                                    stp[:, j, :],
                                    lhsT=Mtw[:, j * P:(j + 1) * P],
                                    rhs=strg[:, w * NH:(w + 1) * NH],
                                    start=True, stop=True)
                            nc.vector.tensor_copy(
                                st_sb[:, boff[w]:boff[w + 1], :],
                                stp[:, 0:ECS[w], :])
                        yield mk
                # ... per-window s_trg + skip from hfmt (needs tiles 0..2)
                def gen_loop1():
                    for w in range(W):
                        def mk(w=w):
                            stp = psd.tile([P, NH + OC], F32, tag="misc")
                            nc.tensor.matmul(stp[:],
                                             lhsT=hfmt[:, w * P:(w + 1) * P],
                                             rhs=btws[:], start=True,
                                             stop=True)
                            nc.vector.tensor_copy(
                                strg[:, w * NH:(w + 1) * NH], stp[:, 0:NH])
                            nc.vector.tensor_copy(skips[:, w],
                                                  stp[:, NH:NH + OC])
                        yield mk

                masks_it = gen_masks()
                # loop1/expand can only start after hfmt tiles (t>=3)
                late_its = None
                mask_per_tile = (SEC + NT - 1) // NT

                def drain(it, n):
                    k = 0
                    for f in it:
                        f()
                        k += 1
                        if n is not None and k >= n:
                            break

                # ---------------- phase A ----------------
                for t0 in range(0, NT, CH):
                    t1 = min(t0 + CH, NT)
                    wdc = (t1 - t0) * 512
                    slc = slice(t0 * 512, t0 * 512 + wdc)
                    xa = pax.tile([P, CH * 512], BF, tag="xa")
                    nc.sync.dma_start(xa[:, 0:wdc], xT[0:P, slc])
                    xb = pax.tile([P, CH * 512], BF, tag="xb")
                    nc.sync.dma_start(xb[:, 0:wdc], xT[P:IN, slc])
                    if t0 == 0:
                        # consts go after the first x chunk so the first FFN
                        # matmuls aren't starved behind their HWDGE slots.
                        nc.sync.dma_start(w1a[:], w1T[0:P, :])
                        nc.sync.dma_start(w1b[:], w1T[P:IN, :])
                        nc.sync.dma_start(b1s[:], b1[:])
                        nc.sync.dma_start(w2s[:], w2[:])
                        nc.sync.dma_start(btws[:], btw[:])
                        nc.sync.dma_start(iotas[:], iota1[:])
                        nc.sync.dma_start(iotac[:], iota_c[:])
                        nc.sync.dma_start(eidxs[:], eidx[:])
                        nc.sync.dma_start(etcols[:], etcol[:])
                    stg = pa.tile([P, CH * 4, 256], BF, tag="stg")
                    for t in range(t0, t1):
                        o = (t - t0) * 512
                        hps = psa.tile([P, 512], F32, tag="hps")
                        nc.tensor.matmul(hps[:], lhsT=w1a[:],
                                         rhs=xa[:, o:o + 512],
                                         start=True, stop=False)
                        nc.tensor.matmul(hps[:], lhsT=w1b[:],
                                         rhs=xb[:, o:o + 512],
                                         start=False, stop=True)
                        hsb = pa.tile([P, 512], BF, tag="hsb")
                        nc.scalar.activation(hsb[:], hps[:], AF.Relu,
                                             bias=b1s[:])
                        if t * 512 < TP:
                            w0 = t * 512
                            w1_ = min(TP, (t + 1) * 512)
                            # hfmt == hsb for these tiles: bf16 4x copy on
                            # DVE instead of a second ACT relu pass.
                            nc.vector.tensor_copy(hfmt[:, w0:w1_],
                                                  hsb[:, 0:(w1_ - w0)])
                        for half in range(2):
                            p2 = psb.tile([P, 2, 256], F32, tag="p2")
                            for jj in range(2):
                                j = half * 2 + jj
                                nc.tensor.matmul(
                                    p2[:, jj, :],
                                    lhsT=hsb[:, j * P:(j + 1) * P],
                                    rhs=w2s[:], start=True, stop=True)
                            sgh = stg[:, (t - t0) * 4 + half * 2:
                                      (t - t0) * 4 + half * 2 + 2, :]
                            # proj + s_src are adjacent (cols 0:136): one
                            # copy per half, engine alternating per tile to
                            # balance ACT/DVE.
                            if half == (t & 1):
                                nc.scalar.activation(sgh[:, :, 0:OC + NH],
                                                     p2[:, :, 0:OC + NH],
                                                     AF.Copy)
                            else:
                                nc.vector.tensor_copy(sgh[:, :, 0:OC + NH],
                                                      p2[:, :, 0:OC + NH])
                        if t < W:
                            bcast_piece(t)
                        # interleave deferred mask/expansion work
                        drain(masks_it, mask_per_tile)
                        if t == 2:
                            drain(gen_loop1(), None)
                            late_its = gen_expand()
                        elif late_its is not None:
                            drain(late_its, 1)
                    # merged tabA write per chunk on the ACT HWDGE queue
                    nc.scalar.dma_start(
                        tabA[t0 * 512:t0 * 512 + wdc, :].rearrange(
                            "(j p) f -> p j f", p=P),
                        stg[:, 0:(t1 - t0) * 4, :])
                    if t1 == NT:
                        drain(masks_it, None)
                        if late_its is not None:
                            drain(late_its, None)

            with tc.tile_pool(name="pe2", bufs=5) as pe2, \
                 tc.tile_pool(name="peg", bufs=6) as peg, \
                 tc.tile_pool(name="pse", bufs=2, space="PSUM") as pse:

                # ---------------- loop 2: per-window edge pipeline ----------
                # finalize is deferred one iteration so the late ELU chain of
                # window w doesn't block window w+1's early ops in the
                # in-order ACT/DVE streams.
                def finalize(w, segp):
                    # every selected target has >=1 in-edge, so the exp-sum
                    # denominator is strictly positive: reciprocal directly
                    # from PSUM, no epsilon pass.
                    rec = pe2.tile([P, 1, NH], F32, tag="rec")
                    nc.vector.reciprocal(rec[:, 0, :], segp[:, OC:OC + NH])
                    z = pe2.tile([P, OC], F32, tag="z")
                    recb = rec[:].broadcast_to([P, HD, NH])
                    nc.vector.tensor_tensor(
                        z[:].rearrange("p (d a) -> p d a", a=NH),
                        segp[:, 0:OC].rearrange("p (d a) -> p d a", a=NH),
                        recb, OP.mult)
                    zs = pe2.tile([P, OC], BF, tag="zs")
                    nc.vector.tensor_add(zs[:], z[:], skips[:, w])
                    # elu: (max(z,0)-1) + exp(min(z,0))
                    am = pe2.tile([P, OC], BF, tag="am")
                    nc.vector.tensor_scalar(am[:], zs[:], 0.0, -1.0, OP.max,
                                            OP.add)
                    bm = pe2.tile([P, OC], BF, tag="bm")
                    nc.vector.tensor_scalar(bm[:], zs[:], 0.0, None, OP.min)
                    eb = pe2.tile([P, OC], BF, tag="eb")
                    nc.scalar.activation(eb[:], bm[:], AF.Exp)
                    fo = pe2.tile([P, OC], BF, tag="fo")
                    nc.vector.tensor_add(fo[:], am[:], eb[:])
                    nc.scalar.dma_start(out[w * P:(w + 1) * P, :], fo[:])

                # each window is processed in two half-pieces: halves the
                # per-piece dependency chain so the pipeline drains ~2x
                # faster after the last gather.
                EHMAX = (ECMAX + 1) // 2
                pending = []
                for w in range(W if PARTS >= 2 else 0):
                    EC = ECS[w]
                    eh0 = (EC + 1) // 2
                    segp = pse.tile([P, 136], F32, tag="segp")
                    for hf, (jb, je) in enumerate([(0, eh0), (eh0, EC)]):
                        if (hf == 1 and w == W - 1 and pending
                                and PARTS >= 4):
                            # last window: finalize the previous window while
                            # the final gather is still in flight.
                            finalize(*pending.pop(0))
                        nj = je - jb
                        if nj == 0:
                            continue
                        G = peg.tile([P, EHMAX, 256], BF, tag="G")
                        nc.gpsimd.dma_gather(
                            G[:, 0:nj, :], tabA[:],
                            eidxs[:, (coff[w] + jb * P) // 16:
                                  (coff[w] + je * P) // 16],
                            nj * P, nj * P, 256, single_packet=False)

                        if PARTS < 3:
                            continue
                        # scores = s_src(gathered) + s_trg(expanded)
                        sc = pe2.tile([P, EHMAX, NH], BF, tag="sc")
                        nc.vector.tensor_tensor(sc[:, 0:nj],
                                                st_sb[:, boff[w] + jb:
                                                      boff[w] + je],
                                                G[:, 0:nj, OC:OC + NH],
                                                OP.add)
                        # exp(leakyrelu(s)) = max(exp(s), exp(0.2 s))
                        e1 = pe2.tile([P, EHMAX, NH], BF, tag="e1")
                        nc.scalar.activation(e1[:, 0:nj], sc[:, 0:nj], AF.Exp)
                        e2 = pe2.tile([P, EHMAX, NH], BF, tag="e2")
                        nc.scalar.activation(e2[:, 0:nj], sc[:, 0:nj], AF.Exp,
                                             scale=0.2)
                        emax = pe2.tile([P, EHMAX, 1, NH], BF, tag="emax")
                        nc.vector.tensor_max(emax[:, 0:nj, 0, :], e1[:, 0:nj],
                                             e2[:, 0:nj])
                        Wv = pe2.tile([P, EHMAX, 136], BF, tag="Wv")
                        nc.scalar.activation(Wv[:, 0:nj, OC:OC + NH],
                                             emax[:, 0:nj, 0, :], AF.Copy)
                        # proj is d-major, so the per-head exp broadcast is a
                        # step-1 inner AP: one packed 2x bf16 multiply, no
                        # materialized expansion.
                        nc.vector.tensor_tensor(
                            Wv[:, 0:nj, 0:OC].rearrange(
                                "p j (d a) -> p j d a", a=NH),
                            G[:, 0:nj, 0:OC].rearrange(
                                "p j (d a) -> p j d a", a=NH),
                            emax[:, 0:nj].broadcast_to([P, nj, HD, NH]),
                            OP.mult)

                        if PARTS < 4:
                            continue
                        for j in range(jb, je):
                            nc.tensor.matmul(
                                segp[:],
                                lhsT=Mw[:, (boff[w] + j) * P:
                                        (boff[w] + j + 1) * P],
                                rhs=Wv[:, j - jb, :], start=(j == 0),
                                stop=(j == EC - 1))
                    if PARTS < 4:
                        continue
                    pending.append((w, segp))
                    if len(pending) > 1:
                        finalize(*pending.pop(0))
                while pending and PARTS >= 4:
                    finalize(*pending.pop(0))

    nc.compile()
    _BUILD_CACHE[key] = nc
    return nc


# ----------------------------------------------------------------------------
# entry point
# ----------------------------------------------------------------------------

def kernel(x, adj0, index0, w_in, b_in, w_proj, a_src, a_trg, w_skip):
    from concourse.bass_utils import run_bass_kernel_spmd

    per_core, NPAD, ECS = plan(x, adj0, index0)
    wts = make_weights(w_in, b_in, w_proj, a_src, a_trg, w_skip)
    nc = build(NPAD, ECS)

    in_maps = []
    for c in range(CORES):
        m = dict(wts)
        m.update({k: v for k, v in per_core[c].items() if k != "inv_k"})
        in_maps.append(m)

    res = run_bass_kernel_spmd(nc, in_maps, core_ids=list(range(CORES)))
    outs = []
    for c, r in enumerate(res.results):
        o = np.asarray(r["out"])[per_core[c]["inv_k"]].astype(np.float32)
        std = np.empty_like(o)
        std[:, COLMAP] = o          # undo the d-major column permutation
        outs.append(std)
    return np.concatenate(outs, axis=0)
